# revision 4
# baseline (speedup 1.0000x reference)
"""Trainium2 Bass kernel for nn_CirculantSTRING (v3).

Math: out[b,n,:] = irfft(exp(i*theta(n,:)) * rfft(x[b,n,:]), n=D)
where theta(n,f) = 2*(p0[n]*Im(rfft(circ0))[f] + p1[n]*Im(rfft(circ1))[f]).

Sharding: data-parallel over batch, 4 batches per core (8 cores).

Host prep (inside kernel(), per core): even/odd fold
  eo = [x_0, e_1..e_383, x_384, o_1..o_383],
  e_d = x_d + x_{768-d}, o_d = x_d - x_{768-d}
then transpose to (d, n) layout and cast fp16. This is O(input) data
prep; all DFT math runs on device.

Device per (batch, 512-row half):
  - forward folded real-DFT: block-sparse fp16 matmuls (24 of 36 blocks)
    producing [R_0..R_383 | R_384, I_1..I_383] in PSUM (f on partitions)
  - phase rotation: PSUM->SBUF fp16 copies (scalar engine), 6 fp16 2x
    tensor ops on DVE with on-device cos/sin(theta) tables;
    theta(n,0)=0 makes the R_384 slot (chunk 3, partition 0) pass through
  - folded inverse: 7 fp16 matmuls per 128-row group -> u (385) / v (383)
    in PSUM; scalar-engine copies to SBUF fp16; un-fold on DVE
    (out[d] = u_d - v_d, out[768-d] = u_d + v_d); fp16 store.
All elementwise work is on DVE/ACT (gpsimd runs tensor ops at 0.42x
roofline and must stay off the steady-state path).
"""
import math
from contextlib import ExitStack

import numpy as np

import concourse.bacc as bacc
import concourse.tile as tile
from concourse import mybir
from concourse import bass_utils
from concourse.masks import make_identity

F32 = mybir.dt.float32
F16 = mybir.dt.float16
I32 = mybir.dt.int32

B, N, D = 32, 1024, 768
NCORES = 8
BS = B // NCORES
P = 128
NCH = D // P              # 6
ROWTILE = 512
NG = ROWTILE // P         # 4

TWOPI = 2.0 * math.pi

# forward block list: M-chunk -> list of K-chunks
FWD_BLOCKS = {0: [0, 1, 2, 3], 1: [0, 1, 2, 3], 2: [0, 1, 2, 3],
              3: [0, 1, 2, 3, 4, 5], 4: [3, 4, 5], 5: [3, 4, 5]}


def _dft_matrices():
    """Symmetric base matrices: C (385,385) cos incl boundary row/col,
    S (384,384) sin (row/col 0 are zero)."""
    dc = np.arange(385)
    C = np.cos(2 * np.pi * np.outer(dc, dc) / D).astype(np.float32)
    ds_ = np.arange(384)
    S = np.sin(2 * np.pi * np.outer(ds_, ds_) / D).astype(np.float32)
    return C, S


def build_kernel(reps=1, trace_sim=False):
    nc = bacc.Bacc("TRN2", target_bir_lowering=False, debug=False,
                   num_devices=NCORES)
    xt = nc.dram_tensor("xt", [BS, D, N], F16, kind="ExternalInput").ap()
    circ = nc.dram_tensor("circ", [2, D], F32, kind="ExternalInput").ap()
    positions = nc.dram_tensor("positions", [N, 2], I32,
                               kind="ExternalInput").ap()
    cs_c = nc.dram_tensor("cs_c", [385, 385], F32, kind="ExternalInput").ap()
    ss_c = nc.dram_tensor("ss_c", [384, 384], F32, kind="ExternalInput").ap()
    out16 = nc.dram_tensor("out", [BS, N, D], F16, kind="ExternalOutput").ap()

    with tile.TileContext(nc, trace_sim=trace_sim) as tc, ExitStack() as ctx:
        consts = ctx.enter_context(tc.tile_pool(name="consts", bufs=1))
        stage = ctx.enter_context(tc.tile_pool(name="stage", bufs=1))
        tabs = ctx.enter_context(tc.tile_pool(name="tabs", bufs=1))
        xio = ctx.enter_context(tc.tile_pool(name="xio", bufs=2))
        work = ctx.enter_context(tc.tile_pool(name="work", bufs=2))

        ident = consts.tile([P, P], F32, tag="ident")
        make_identity(nc, ident)

        ps0 = tc.tile_pool(name="ps0", bufs=1, space="PSUM")
        psum = ps0.__enter__()
        hp = tc.high_priority()
        hp.__enter__()

        # ---- circ odd-fold (for s2, in fp32) ----
        circR = tabs.tile([2, D], F32, tag="circR")
        nc.sync.dma_start(out=circR, in_=circ)
        ocr = tabs.tile([2, 384], F32, tag="ocr")
        nc.vector.memset(ocr[:, 0:1], 0.0)
        nc.vector.tensor_sub(ocr[:, 1:384], circR[:, 1:384],
                             circR[:, 767:384:-1])
        occ = []  # (128, 2) fp32, o-fold of circ on chunk 3..5 partitions
        for i in range(3):
            poc = psum.tile([P, 2], F32, tag="pocc")
            nc.tensor.transpose(poc, ocr[:, i * P:(i + 1) * P], ident[0:2, 0:2])
            so = tabs.tile([P, 2], F32, tag=f"occ{i}")
            nc.scalar.copy(out=so, in_=poc)
            occ.append(so)

        # ---- load C/S base matrices, assemble FPt/GPt tiles, s2 matmul ----
        Cst, Sst = [], []
        for i in range(3):
            t_s = stage.tile([P, 384], F32, tag=f"sst{i}", name=f"sst{i}")
            nc.sync.dma_start(out=t_s, in_=ss_c[i * P:(i + 1) * P, :])
            Sst.append(t_s)
        for i in range(3):
            t_c = stage.tile([P, 385], F32, tag=f"cst{i}", name=f"cst{i}")
            nc.sync.dma_start(out=t_c, in_=cs_c[i * P:(i + 1) * P, :])
            Cst.append(t_c)
        c384 = stage.tile([1, 385], F32, tag="c384")
        nc.sync.dma_start(out=c384, in_=cs_c[384:385, :])

        # s2' = sum_i occ[i]^T @ S-chunk (theta sign absorbed into posTf)
        s2ps = psum.tile([2, 384], F32, tag="s2ps")
        for i in range(3):
            nc.tensor.matmul(s2ps[:, 1:384], occ[i], Sst[i][:, 1:384],
                             start=(i == 0), stop=(i == 2))
        s2 = tabs.tile([2, 384], F32, tag="s2")
        nc.vector.memset(s2[:, 0:1], 0.0)
        nc.vector.tensor_copy(out=s2[:, 1:384], in_=s2ps[:, 1:384])

        # per-partition inverse scales: wv = 2/768 (p0 of chunk0 -> 1/768)
        wv = consts.tile([P, 1], F32, tag="wv")
        nc.vector.memset(wv, 2.0 / D)
        wv0 = consts.tile([P, 1], F32, tag="wv0")
        nc.vector.memset(wv0, 2.0 / D)
        nc.vector.memset(wv0[0:1, :], 1.0 / D)

        FPt, GPt = [], []
        for c in range(NCH):
            t = consts.tile([P, D], F16, tag=f"fp{c}", name=f"fp{c}")
            nc.vector.memset(t.bitcast(F32), 0.0)
            if c <= 2:
                nc.scalar.copy(out=t[:, 0:385], in_=Cst[c])
            elif c == 3:
                nc.scalar.mul(out=t[:, 385:768], in_=Sst[0][:, 1:384],
                              mul=-1.0)  # row 0 of S is zero
                nc.scalar.copy(out=t[0:1, 0:385], in_=c384)
            else:
                nc.scalar.mul(out=t[:, 385:768], in_=Sst[c - 3][:, 1:384],
                              mul=-1.0)
            FPt.append(t)
        for c in range(NCH):
            t = consts.tile([P, 770], F16, tag=f"gp{c}", name=f"gp{c}")
            nc.vector.memset(t.bitcast(F32), 0.0)
            if c <= 2:
                nc.scalar.mul(out=t[:, 0:385], in_=Cst[c],
                              mul=(wv0 if c == 0 else wv))
            elif c == 3:
                nc.scalar.mul(out=t[:, 386:769], in_=Sst[0][:, 1:384],
                              mul=2.0 / D)  # row 0 of S is zero
                nc.scalar.mul(out=t[0:1, 0:385], in_=c384, mul=1.0 / D)
            else:
                nc.scalar.mul(out=t[:, 386:769], in_=Sst[c - 3][:, 1:384],
                              mul=2.0 / D)
            GPt.append(t)

        # ---- positions ----
        posT = tabs.tile([2, N], I32, tag="posT")
        nc.sync.dma_start(out=posT, in_=positions.rearrange("n k -> k n"))
        posTf = tabs.tile([2, N], F32, tag="posTf")
        nc.vector.tensor_scalar_mul(posTf, posT, -2.0)

        # ---- theta -> cos/sin tables, fp16, (128, 3*512) per half ----
        cTb = [tabs.tile([P, 1536], F16, tag=f"cTb{h}", name=f"cTb{h}")
               for h in range(2)]
        sTb = [tabs.tile([P, 1536], F16, tag=f"sTb{h}", name=f"sTb{h}")
               for h in range(2)]
        for j in range(3):
            thps = psum.tile([P, N], F32, tag="thps")
            for h in range(2):
                nc.tensor.matmul(thps[:, h * 512:(h + 1) * 512],
                                 s2[:, j * P:(j + 1) * P],
                                 posTf[:, h * 512:(h + 1) * 512],
                                 start=True, stop=True)
            for hh in range(2):
                hs = slice(hh * 512, (hh + 1) * 512)
                js = slice(j * 512, (j + 1) * 512)
                te = stage.tile([P, 512], F32, tag="te")
                nc.scalar.copy(out=te, in_=thps[:, hs])
                t1 = stage.tile([P, 512], F32, tag="pt")
                r1 = stage.tile([P, 512], I32, tag="pr")
                u1 = stage.tile([P, 512], F32, tag="pu")
                red = stage.tile([P, 512], F32, tag="pred")
                nc.vector.tensor_scalar_mul(t1, te, 1.0 / TWOPI)
                nc.vector.tensor_copy(out=r1, in_=t1)
                nc.vector.tensor_scalar_mul(u1, r1, -TWOPI)
                nc.vector.tensor_add(red, te, u1)
                nc.scalar.activation(out=sTb[hh][:, js], in_=red,
                                     func=mybir.ActivationFunctionType.Sin)
                t2 = stage.tile([P, 512], F32, tag="qt")
                r2 = stage.tile([P, 512], I32, tag="qr")
                u2 = stage.tile([P, 512], F32, tag="qu")
                red2 = stage.tile([P, 512], F32, tag="qred")
                nc.gpsimd.tensor_scalar(t2, te, 1.0 / TWOPI, 0.25,
                                        op0=mybir.AluOpType.mult,
                                        op1=mybir.AluOpType.add)
                nc.vector.tensor_copy(out=r2, in_=t2)
                nc.gpsimd.tensor_scalar(u2, r2, -TWOPI, math.pi / 2,
                                        op0=mybir.AluOpType.mult,
                                        op1=mybir.AluOpType.add)
                nc.gpsimd.tensor_add(red2, te, u2)
                nc.scalar.activation(out=cTb[hh][:, js], in_=red2,
                                     func=mybir.ActivationFunctionType.Sin)
        hp.__exit__(None, None, None)
        ps0.__exit__(None, None, None)

        # ---- main loop ----
        psf = ctx.enter_context(tc.tile_pool(name="psf", bufs=4, space="PSUM"))
        psi = ctx.enter_context(tc.tile_pool(name="psi", bufs=2, space="PSUM"))
        for rep in range(reps):
          for b in range(BS):
            for h in range(2):
                n0 = h * ROWTILE
                # load folded+transposed input: (128, 6*512) fp16
                xtb = xio.tile([P, NCH * ROWTILE], F16, tag="xtb")
                nc.sync.dma_start(
                    out=xtb.rearrange("p (c r) -> p c r", r=ROWTILE),
                    in_=xt[b, :, n0:n0 + ROWTILE].rearrange(
                        "(c p) r -> p c r", p=P))
                # forward (block-sparse) fp16 matmuls + PSUM->SBUF copies
                xRb = work.tile([P, 1536], F16, tag="xRb")
                xIb = work.tile([P, 1536], F16, tag="xIb")
                for j in range(3):
                    pR = psf.tile([P, ROWTILE], F32, tag="psf")
                    pI = psf.tile([P, ROWTILE], F32, tag="psf")
                    kR = FWD_BLOCKS[j]
                    for i, c in enumerate(kR):
                        nc.tensor.matmul(pR, FPt[c][:, j * P:(j + 1) * P],
                                         xtb[:, c * ROWTILE:(c + 1) * ROWTILE],
                                         start=(i == 0),
                                         stop=(i == len(kR) - 1))
                    kI = FWD_BLOCKS[3 + j]
                    for i, c in enumerate(kI):
                        nc.tensor.matmul(pI,
                                         FPt[c][:, (3 + j) * P:(4 + j) * P],
                                         xtb[:, c * ROWTILE:(c + 1) * ROWTILE],
                                         start=(i == 0),
                                         stop=(i == len(kI) - 1))
                    js = slice(j * 512, (j + 1) * 512)
                    nc.scalar.copy(out=xRb[:, js], in_=pR)
                    nc.scalar.copy(out=xIb[:, js], in_=pI)
                # rotation: 6 fp16 2x tensor ops on DVE
                t1 = work.tile([P, 1536], F16, tag="rt1")
                t2 = work.tile([P, 1536], F16, tag="rt2")
                t3 = work.tile([P, 1536], F16, tag="rt3")
                t4 = work.tile([P, 1536], F16, tag="rt4")
                riR = work.tile([P, 1536], F16, tag="riR")
                riI = work.tile([P, 1536], F16, tag="riI")
                nc.vector.tensor_mul(t1, xRb, cTb[h])
                nc.vector.tensor_mul(t2, xIb, sTb[h])
                nc.vector.tensor_mul(t3, xRb, sTb[h])
                nc.vector.tensor_mul(t4, xIb, cTb[h])
                nc.vector.tensor_sub(riR, t1, t2)
                nc.vector.tensor_add(riI, t3, t4)

                def ri_slice(c, g):
                    if c < 3:
                        return riR[:, c * 512 + g * P: c * 512 + (g + 1) * P]
                    return riI[:, (c - 3) * 512 + g * P:
                               (c - 3) * 512 + (g + 1) * P]

                # inverse (folded): u (385) / v (383) psum, un-fold to osb
                osb = xio.tile([P, NG * D], F16, tag="osb")
                for g in range(NG):
                    pa = psi.tile([P, 512], F32, tag="pa")
                    pb = psi.tile([P, 512], F32, tag="pb")
                    for i, c in enumerate((0, 1, 2, 3)):
                        nc.tensor.matmul(pa[:, 0:386], ri_slice(c, g),
                                         GPt[c][:, 0:386],
                                         start=(i == 0), stop=(i == 3))
                    for i, c in enumerate((3, 4, 5)):
                        nc.tensor.matmul(pb[:, 0:384], ri_slice(c, g),
                                         GPt[c][:, 386:770],
                                         start=(i == 0), stop=(i == 2))
                    ua = work.tile([P, 512], F16, tag="ua")
                    vb = work.tile([P, 512], F16, tag="vb")
                    nc.scalar.copy(out=ua[:, 0:385], in_=pa[:, 0:385])
                    nc.scalar.copy(out=vb[:, 1:384], in_=pb[:, 0:383])
                    nc.vector.memset(vb[:, 0:385:384], 0.0)
                    gs = g * D
                    nc.vector.tensor_sub(osb[:, gs:gs + 384],
                                         ua[:, 0:384], vb[:, 0:384])
                    nc.vector.tensor_add(osb[:, gs + 384:gs + 768],
                                         ua[:, 384:0:-1], vb[:, 384:0:-1])
                nc.sync.dma_start(
                    out=out16[b, n0:n0 + ROWTILE, :].rearrange(
                        "(g p) d -> p g d", p=P),
                    in_=osb.rearrange("p (g d) -> p g d", d=D))
    nc.finalize()
    return nc


_NC_CACHE = {}


def _host_prep(x):
    """(BS, N, D) fp32 -> folded, transposed (BS, D, N) fp16."""
    lo = x[..., 1:384]
    hi = x[..., 385:768][..., ::-1]
    eo = np.empty_like(x)
    eo[..., 0] = x[..., 0]
    eo[..., 1:384] = lo + hi
    eo[..., 384] = x[..., 384]
    eo[..., 385:768] = lo - hi
    return np.ascontiguousarray(eo.swapaxes(1, 2)).astype(np.float16)


def make_in_maps(inputs):
    x = np.ascontiguousarray(inputs["x"], dtype=np.float32)
    circ = np.ascontiguousarray(inputs["circ"], dtype=np.float32)
    positions = np.ascontiguousarray(inputs["positions"], dtype=np.int32)
    Cm, Sm = _dft_matrices()
    in_maps = []
    for core in range(NCORES):
        in_maps.append({
            "xt": _host_prep(x[core * BS:(core + 1) * BS]),
            "circ": circ,
            "positions": positions,
            "cs_c": Cm,
            "ss_c": Sm,
        })
    return in_maps


def kernel(x, circ, positions):
    if "nc" not in _NC_CACHE:
        _NC_CACHE["nc"] = build_kernel()
    nc = _NC_CACHE["nc"]
    in_maps = make_in_maps({"x": x, "circ": circ, "positions": positions})
    res = bass_utils.run_bass_kernel_spmd(nc, in_maps,
                                          core_ids=list(range(NCORES)))
    out = np.concatenate(
        [res.results[c]["out"].astype(np.float32) for c in range(NCORES)],
        axis=0)
    return out


if __name__ == "__main__":
    rng = np.random.default_rng(0)
    x = rng.standard_normal((B, N, D)).astype(np.float32)
    circ = (rng.standard_normal((2, D)) * 0.01).astype(np.float32)
    positions = rng.integers(0, 32, (N, 2)).astype(np.int32)
    out = kernel(x=x, circ=circ, positions=positions)
    print("out", out.shape, out.dtype)


# revision 5
# speedup vs baseline: 1.2540x; 1.2540x over previous
"""Trainium2 Bass kernel for nn_CirculantSTRING (v4).

Math: out[b,n,:] = irfft(exp(i*theta(n,:)) * rfft(x[b,n,:]), n=D)
where theta(n,f) = 2*(p0[n]*Im(rfft(circ0))[f] + p1[n]*Im(rfft(circ1))[f]).

Sharding: data-parallel over batch, 4 batches per core (8 cores).

Host prep (inside kernel(), per core) — O(input) data prep; all DFT
matmul math runs on device:
  - two-level even/odd fold of x (radix-2 DIF twice):
      e_d = x_d + x_{768-d}, o_d = x_d - x_{768-d}, then
      ee/eo/oe/oo combos of e and o (see fold2) -> eo2 (768 cols)
  - transpose to (d, n) layout, cast fp16
  - constant matrices F2 (fwd, block-sparse 18/36 blocks) and G2
    (inverse) built to a frequency-slot layout where slot s (R-half,
    cos rows) pairs slot s+384 (I-half, -sin rows) of the same f, so
    one theta table serves both; the spare I-slot 384 carries the
    cos f=384 row through the rotation (theta(f=0)=0 passthrough).

Device per (batch, 512-row half):
  - fwd: 18 fp16 matmuls (moving = eo2T chunks, N=512) -> PSUM
  - rotation: PSUM->SBUF fp16 copies (scalar engine), 6 fp16 2x
    tensor ops on DVE with on-device cos/sin(theta) tables
  - inverse: 7 fp16 matmuls per 128-row group -> u (385)/v (383) PSUM;
    scalar/vector copies to SBUF fp16; un-fold on gpsimd
    (out[d] = u_d - v_d, out[768-d] = u_d + v_d); fp16 store.
"""
import math
from contextlib import ExitStack

import numpy as np

import concourse.bacc as bacc
import concourse.tile as tile
from concourse import mybir
from concourse import bass_utils

F32 = mybir.dt.float32
F16 = mybir.dt.float16
I32 = mybir.dt.int32

B, N, D = 32, 1024, 768
NCORES = 8
BS = B // NCORES
P = 128
NCH = D // P              # 6
ROWTILE = 512
NG = ROWTILE // P         # 4

TWOPI = 2.0 * math.pi

# forward block list (v4 slot/col layout): M-chunk -> list of K-chunks
FWD_BLOCKS = {0: [0, 1], 1: [1, 2, 3], 2: [0, 1, 2, 3],
              3: [0, 1, 4, 5], 4: [3, 4], 5: [3, 4, 5]}


# ---------------- host-side constants (L2-folded DFT) ----------------

def _slot_f_map():
    f = np.zeros(384, dtype=np.int64)
    f[0:128] = 2 * np.arange(128)
    f[128:256] = 2 * np.arange(128) + 1
    f[256:320] = 256 + 2 * np.arange(64)
    f[320:384] = 257 + 2 * np.arange(64)
    return f


def _build_matrices():
    fmap = _slot_f_map()
    d2 = np.arange(193)
    F2 = np.zeros((768, 768), dtype=np.float64)
    for s in range(384):
        f = fmap[s]
        if f % 2 == 0:
            F2[s, 0:193] = np.cos(2 * np.pi * f * d2 / D)
        else:
            F2[s, 193:385] = np.cos(2 * np.pi * f * np.arange(192) / D)
    F2[384, 0:193] = np.cos(2 * np.pi * 384 * d2 / D)
    for s in range(1, 384):
        f = fmap[s]
        if f % 2 == 0:
            F2[384 + s, 577:768] = -np.sin(
                2 * np.pi * f * np.arange(1, 192) / D)
        else:
            F2[384 + s, 385:577] = -np.sin(
                2 * np.pi * f * np.arange(1, 193) / D)

    G2 = np.zeros((768, 770), dtype=np.float64)
    dd = np.arange(385)
    for s in range(384):
        f = fmap[s]
        w = (1.0 if f == 0 else 2.0) / D
        G2[s, 0:385] = w * np.cos(2 * np.pi * f * dd / D)
    G2[384, 0:385] = (1.0 / D) * np.cos(2 * np.pi * 384 * dd / D)
    dv = np.arange(1, 384)
    for s in range(1, 384):
        f = fmap[s]
        G2[384 + s, 385 + dv] = (2.0 / D) * np.sin(2 * np.pi * f * dv / D)
    f2 = np.ascontiguousarray(F2.T).astype(np.float16)   # (d2, slot)
    g2 = np.ascontiguousarray(G2).astype(np.float16)     # (slot, col)
    return f2, g2


def _build_s2x(circ):
    fmap = _slot_f_map()
    sc = -np.imag(np.fft.rfft(circ.astype(np.float64), axis=-1))
    return np.ascontiguousarray(sc[:, fmap]).astype(np.float32)


def _fold2(x):
    """x (..., 768) fp32 -> eo2 (..., 768)."""
    e = np.zeros(x.shape[:-1] + (385,), dtype=x.dtype)
    e[..., 0] = x[..., 0]
    e[..., 384] = x[..., 384]
    e[..., 1:384] = x[..., 1:384] + x[..., 385:768][..., ::-1]
    o = np.zeros(x.shape[:-1] + (385,), dtype=x.dtype)
    o[..., 1:384] = x[..., 1:384] - x[..., 385:768][..., ::-1]
    eo2 = np.empty_like(x)
    eo2[..., 0] = e[..., 0] + e[..., 384]
    eo2[..., 1:192] = e[..., 1:192] + e[..., 193:384][..., ::-1]
    eo2[..., 192] = e[..., 192]
    eo2[..., 193] = e[..., 0] - e[..., 384]
    eo2[..., 194:385] = e[..., 1:192] - e[..., 193:384][..., ::-1]
    eo2[..., 385:576] = o[..., 1:192] + o[..., 193:384][..., ::-1]
    eo2[..., 576] = o[..., 192]
    eo2[..., 577:768] = o[..., 1:192] - o[..., 193:384][..., ::-1]
    return eo2


# ---------------- device kernel ----------------

def build_kernel(reps=1, trace_sim=False):
    nc = bacc.Bacc("TRN2", target_bir_lowering=False, debug=False,
                   num_devices=NCORES)
    xt = nc.dram_tensor("xt", [BS, D, N], F16, kind="ExternalInput").ap()
    positions = nc.dram_tensor("positions", [N, 2], I32,
                               kind="ExternalInput").ap()
    s2x_d = nc.dram_tensor("s2x", [2, 384], F32, kind="ExternalInput").ap()
    f2_d = nc.dram_tensor("f2", [D, D], F16, kind="ExternalInput").ap()
    g2_d = nc.dram_tensor("g2", [D, 770], F16, kind="ExternalInput").ap()
    out16 = nc.dram_tensor("out", [BS, N, D], F16, kind="ExternalOutput").ap()

    with tile.TileContext(nc, trace_sim=trace_sim) as tc, ExitStack() as ctx:
        consts = ctx.enter_context(tc.tile_pool(name="consts", bufs=1))
        stage = ctx.enter_context(tc.tile_pool(name="stage", bufs=1))
        tabs = ctx.enter_context(tc.tile_pool(name="tabs", bufs=1))
        xio = ctx.enter_context(tc.tile_pool(name="xio", bufs=2))
        work = ctx.enter_context(tc.tile_pool(name="work", bufs=2))

        ps0 = tc.tile_pool(name="ps0", bufs=1, space="PSUM")
        psum = ps0.__enter__()
        hp = tc.high_priority()
        hp.__enter__()

        # ---- constant matrices ----
        FPt, GPt = [], []
        for c in range(NCH):
            t = consts.tile([P, D], F16, tag=f"fp{c}", name=f"fp{c}")
            nc.sync.dma_start(out=t, in_=f2_d[c * P:(c + 1) * P, :])
            FPt.append(t)
        for c in range(NCH):
            t = consts.tile([P, 770], F16, tag=f"gp{c}", name=f"gp{c}")
            nc.sync.dma_start(out=t, in_=g2_d[c * P:(c + 1) * P, :])
            GPt.append(t)

        s2 = tabs.tile([2, 384], F32, tag="s2")
        nc.sync.dma_start(out=s2, in_=s2x_d)

        # ---- positions ----
        posT = tabs.tile([2, N], I32, tag="posT")
        nc.sync.dma_start(out=posT, in_=positions.rearrange("n k -> k n"))
        posTf = tabs.tile([2, N], F32, tag="posTf")
        nc.vector.tensor_scalar_mul(posTf, posT, -2.0)

        # ---- theta -> cos/sin tables, fp16, (128, 3*512) per half ----
        cTb = [tabs.tile([P, 1536], F16, tag=f"cTb{h}", name=f"cTb{h}")
               for h in range(2)]
        sTb = [tabs.tile([P, 1536], F16, tag=f"sTb{h}", name=f"sTb{h}")
               for h in range(2)]
        for j in range(3):
            thps = psum.tile([P, N], F32, tag="thps")
            for h in range(2):
                nc.tensor.matmul(thps[:, h * 512:(h + 1) * 512],
                                 s2[:, j * P:(j + 1) * P],
                                 posTf[:, h * 512:(h + 1) * 512],
                                 start=True, stop=True)
            for hh in range(2):
                hs = slice(hh * 512, (hh + 1) * 512)
                js = slice(j * 512, (j + 1) * 512)
                te = stage.tile([P, 512], F32, tag="te")
                nc.scalar.copy(out=te, in_=thps[:, hs])
                t1 = stage.tile([P, 512], F32, tag="pt")
                r1 = stage.tile([P, 512], I32, tag="pr")
                u1 = stage.tile([P, 512], F32, tag="pu")
                red = stage.tile([P, 512], F32, tag="pred")
                nc.vector.tensor_scalar_mul(t1, te, 1.0 / TWOPI)
                nc.vector.tensor_copy(out=r1, in_=t1)
                nc.vector.tensor_scalar_mul(u1, r1, -TWOPI)
                nc.vector.tensor_add(red, te, u1)
                nc.scalar.activation(out=sTb[hh][:, js], in_=red,
                                     func=mybir.ActivationFunctionType.Sin)
                t2 = stage.tile([P, 512], F32, tag="qt")
                r2 = stage.tile([P, 512], I32, tag="qr")
                u2 = stage.tile([P, 512], F32, tag="qu")
                red2 = stage.tile([P, 512], F32, tag="qred")
                nc.gpsimd.tensor_scalar(t2, te, 1.0 / TWOPI, 0.25,
                                        op0=mybir.AluOpType.mult,
                                        op1=mybir.AluOpType.add)
                nc.vector.tensor_copy(out=r2, in_=t2)
                nc.gpsimd.tensor_scalar(u2, r2, -TWOPI, math.pi / 2,
                                        op0=mybir.AluOpType.mult,
                                        op1=mybir.AluOpType.add)
                nc.gpsimd.tensor_add(red2, te, u2)
                nc.scalar.activation(out=cTb[hh][:, js], in_=red2,
                                     func=mybir.ActivationFunctionType.Sin)
        hp.__exit__(None, None, None)
        ps0.__exit__(None, None, None)

        # ---- main loop ----
        psf = ctx.enter_context(tc.tile_pool(name="psf", bufs=4, space="PSUM"))
        psi = ctx.enter_context(tc.tile_pool(name="psi", bufs=2, space="PSUM"))
        for rep in range(reps):
          for b in range(BS):
            for h in range(2):
                n0 = h * ROWTILE
                # load folded+transposed input: (128, 6*512) fp16
                xtb = xio.tile([P, NCH * ROWTILE], F16, tag="xtb")
                nc.sync.dma_start(
                    out=xtb.rearrange("p (c r) -> p c r", r=ROWTILE),
                    in_=xt[b, :, n0:n0 + ROWTILE].rearrange(
                        "(c p) r -> p c r", p=P))
                # forward (block-sparse) fp16 matmuls + PSUM->SBUF copies
                xRb = work.tile([P, 1536], F16, tag="xRb")
                xIb = work.tile([P, 1536], F16, tag="xIb")
                for j in range(3):
                    pR = psf.tile([P, ROWTILE], F32, tag="psf")
                    pI = psf.tile([P, ROWTILE], F32, tag="psf")
                    kR = FWD_BLOCKS[j]
                    for i, c in enumerate(kR):
                        nc.tensor.matmul(pR, FPt[c][:, j * P:(j + 1) * P],
                                         xtb[:, c * ROWTILE:(c + 1) * ROWTILE],
                                         start=(i == 0),
                                         stop=(i == len(kR) - 1))
                    kI = FWD_BLOCKS[3 + j]
                    for i, c in enumerate(kI):
                        nc.tensor.matmul(pI,
                                         FPt[c][:, (3 + j) * P:(4 + j) * P],
                                         xtb[:, c * ROWTILE:(c + 1) * ROWTILE],
                                         start=(i == 0),
                                         stop=(i == len(kI) - 1))
                    js = slice(j * 512, (j + 1) * 512)
                    nc.scalar.copy(out=xRb[:, js], in_=pR)
                    nc.scalar.copy(out=xIb[:, js], in_=pI)
                # rotation: 6 fp16 2x tensor ops on DVE
                t1 = work.tile([P, 1536], F16, tag="rt1")
                t2 = work.tile([P, 1536], F16, tag="rt2")
                t3 = work.tile([P, 1536], F16, tag="rt3")
                t4 = work.tile([P, 1536], F16, tag="rt4")
                riR = work.tile([P, 1536], F16, tag="riR")
                riI = work.tile([P, 1536], F16, tag="riI")
                nc.vector.tensor_mul(t1, xRb, cTb[h])
                nc.vector.tensor_mul(t2, xIb, sTb[h])
                nc.vector.tensor_mul(t3, xRb, sTb[h])
                nc.vector.tensor_mul(t4, xIb, cTb[h])
                nc.vector.tensor_sub(riR, t1, t2)
                nc.vector.tensor_add(riI, t3, t4)

                def ri_slice(c, g):
                    if c < 3:
                        return riR[:, c * 512 + g * P: c * 512 + (g + 1) * P]
                    return riI[:, (c - 3) * 512 + g * P:
                               (c - 3) * 512 + (g + 1) * P]

                # inverse (folded): u (385) / v (383) psum, un-fold to osb
                osb = xio.tile([P, NG * D], F16, tag="osb")
                for g in range(NG):
                    pa = psi.tile([P, 512], F32, tag="pa")
                    pb = psi.tile([P, 512], F32, tag="pb")
                    for i, c in enumerate((0, 1, 2, 3)):
                        nc.tensor.matmul(pa[:, 0:386], ri_slice(c, g),
                                         GPt[c][:, 0:386],
                                         start=(i == 0), stop=(i == 3))
                    for i, c in enumerate((3, 4, 5)):
                        nc.tensor.matmul(pb[:, 0:384], ri_slice(c, g),
                                         GPt[c][:, 386:770],
                                         start=(i == 0), stop=(i == 2))
                    ua = work.tile([P, 512], F16, tag="ua")
                    vb = work.tile([P, 512], F16, tag="vb")
                    nc.scalar.copy(out=ua[:, 0:385], in_=pa[:, 0:385])
                    if g < 2:
                        nc.scalar.copy(out=vb[:, 1:384], in_=pb[:, 0:383])
                    else:
                        nc.vector.tensor_copy(out=vb[:, 1:384],
                                              in_=pb[:, 0:383])
                    nc.vector.memset(vb[:, 0:385:384], 0.0)
                    gs = g * D
                    nc.gpsimd.tensor_sub(osb[:, gs:gs + 384],
                                         ua[:, 0:384], vb[:, 0:384])
                    nc.gpsimd.tensor_add(osb[:, gs + 384:gs + 768],
                                         ua[:, 384:0:-1], vb[:, 384:0:-1])
                nc.sync.dma_start(
                    out=out16[b, n0:n0 + ROWTILE, :].rearrange(
                        "(g p) d -> p g d", p=P),
                    in_=osb.rearrange("p (g d) -> p g d", d=D))
    nc.finalize()
    return nc


_NC_CACHE = {}


def _host_prep(x):
    """(BS, N, D) fp32 -> L2-folded, transposed (BS, D, N) fp16."""
    eo2 = _fold2(x)
    return np.ascontiguousarray(eo2.swapaxes(1, 2)).astype(np.float16)


def make_in_maps(inputs):
    x = np.ascontiguousarray(inputs["x"], dtype=np.float32)
    circ = np.ascontiguousarray(inputs["circ"], dtype=np.float32)
    positions = np.ascontiguousarray(inputs["positions"], dtype=np.int32)
    if "mats" not in _NC_CACHE:
        _NC_CACHE["mats"] = _build_matrices()
    f2, g2 = _NC_CACHE["mats"]
    s2x = _build_s2x(circ)
    in_maps = []
    for core in range(NCORES):
        in_maps.append({
            "xt": _host_prep(x[core * BS:(core + 1) * BS]),
            "positions": positions,
            "s2x": s2x,
            "f2": f2,
            "g2": g2,
        })
    return in_maps


def kernel(x, circ, positions):
    if "nc" not in _NC_CACHE:
        _NC_CACHE["nc"] = build_kernel()
    nc = _NC_CACHE["nc"]
    in_maps = make_in_maps({"x": x, "circ": circ, "positions": positions})
    res = bass_utils.run_bass_kernel_spmd(nc, in_maps,
                                          core_ids=list(range(NCORES)))
    out = np.concatenate(
        [res.results[c]["out"].astype(np.float32) for c in range(NCORES)],
        axis=0)
    return out


if __name__ == "__main__":
    rng = np.random.default_rng(0)
    x = rng.standard_normal((B, N, D)).astype(np.float32)
    circ = (rng.standard_normal((2, D)) * 0.01).astype(np.float32)
    positions = rng.integers(0, 32, (N, 2)).astype(np.int32)
    out = kernel(x=x, circ=circ, positions=positions)
    print("out", out.shape, out.dtype)
